# revision 1
# baseline (speedup 1.0000x reference)
# Trainium2 Bass kernel for nn_DifferentiableFeatureLayer.
#
# Math (per reference):
#   bw[b]   = full_series[starts[b]-W : starts[b]+T]            (B, W+T, C)
#   f_mean  = conv(bw, w1)/s1 ; m2 = conv(bw, w2)/s2
#   var2    = conv(bw^2, w2)/s2 - m2^2 ; f_std = sqrt(var2 + 1e-8)
#   out     = concat([x, BN(f_mean), BN(f_std)], -1)            (B, T, 3C)
# where conv is a per-channel sliding window of length W over time and BN
# normalizes per channel over (B, T).
#
# Sharding: by channel — core k owns channels [4k, 4k+4). BN is per channel,
# so every core is fully independent (no collectives). The host extracts the
# B runtime-indexed windows (tiny: 1.25 MB) and passes x through on the host.
#
# Device compute: the sliding window is a banded (Toeplitz) matmul. For
# channel c, pass p in {0,1}:
#   f[b, 128q+r] = sum_p sum_kp T_p[kp, r] * G[kp, b, q+p]
#   T_p[kp, r]   = w[128p + kp - r]  (0 outside [0, 128))   — stationary
#   G[kp, b, j]  = bw[b, 128j + kp]                         — moving
# so each conv is 2 accumulating PE matmuls (K=128, M=128, N=64) per channel.
#
# The mean feature is kept unnormalized (1/s1 folds into the BN affine);
# f_std = sqrt(acc3/s2 - (acc2/s2)^2 + 1e-8) elementwise; BN per channel is
# one fused affine a*f + b with a, b from per-seg (sum, sumsq) reductions.
#
# All inputs arrive in TWO dma_starts (blob1 = G | constants | toeplitz c0/c1,
# blob2 = toeplitz c2/c3); constants are host-replicated across partitions so
# per-partition scalar operands are direct blob column slices.

import numpy as np

import concourse.bass as bass
import concourse.bacc as bacc
import concourse.tile as tile
from concourse import mybir
from concourse.bass_utils import run_bass_kernel_spmd

B, T, C = 16, 512, 32
W = 128
SERIES_LEN = 100000
WIN_MIN, WIN_MAX = 2.0, 64.0
SHARP = 1.0
BN_EPS = 1e-5
STD_EPS = 1e-8

NCORES = 8
CPC = C // NCORES          # channels per core = 4
NSEG = 2 * CPC             # 8 per-core output channels (4 mean + 4 std)
Q = T // 128               # 4 time blocks
NB = B * Q                 # 64 matmul columns
NBT = B * T                # BN population per channel
F32 = mybir.dt.float32
MUL = mybir.AluOpType.mult
ADD = mybir.AluOpType.add
SQRT = mybir.ActivationFunctionType.Sqrt

GW = CPC * B * (Q + 1)     # 320 g columns in blob1
CW = 64                    # constant columns in blob1
TW = 4 * 128               # toeplitz columns per channel (ty, p, r)


def _sigmoid(x):
    out = np.empty_like(x)
    pos = x >= 0
    out[pos] = 1.0 / (1.0 + np.exp(-x[pos]))
    ex = np.exp(x[~pos])
    out[~pos] = ex / (1.0 + ex)
    return out


def _soft_window_weights(raw):
    # (C,) -> (W, C), float64 for host-side accuracy
    win = WIN_MIN + _sigmoid(raw.astype(np.float64)) * (WIN_MAX - WIN_MIN)
    age = np.arange(W, dtype=np.float64)[::-1]
    return _sigmoid(SHARP * (win[None, :] - age[:, None]))


def _toeplitz_pair(wt):
    # wt: (W,) -> (2, 128, 128) band matrices T_p[kp, r] = wt[128p + kp - r]
    kp = np.arange(128)[:, None]
    r = np.arange(128)[None, :]
    out = np.zeros((2, 128, 128), np.float64)
    for p in range(2):
        idx = 128 * p + kp - r
        valid = (idx >= 0) & (idx < W)
        out[p] = np.where(valid, wt[np.clip(idx, 0, W - 1)], 0.0)
    return out


def _build_nc():
    nc = bacc.Bacc("TRN2", target_bir_lowering=False, debug=False,
                   num_devices=NCORES)
    b1_t = nc.dram_tensor("blob1", [128, GW + CW + 2 * TW], F32,
                          kind="ExternalInput")
    b2_t = nc.dram_tensor("blob2", [128, 2 * TW], F32, kind="ExternalInput")
    # out[r, seg, b*Q+q]: fully contiguous per partition for the store DMA
    out_t = nc.dram_tensor("out", [128, NSEG, NB], F32, kind="ExternalOutput")
    b1ap, b2ap, oap = b1_t.ap(), b2_t.ap(), out_t.ap()

    with tile.TileContext(nc) as tc:
        with (
            tc.tile_pool(name="consts", bufs=1) as cpool,
            tc.tile_pool(name="work", bufs=1) as work,
            tc.tile_pool(name="ps1", bufs=2, space="PSUM") as ps1,
            tc.tile_pool(name="ps2", bufs=2, space="PSUM") as ps2,
            tc.tile_pool(name="ps3", bufs=2, space="PSUM") as ps3,
            tc.tile_pool(name="pss", bufs=1, space="PSUM") as pss,
        ):
            ones_c = cpool.tile([128, 1], F32, tag="ones_c")
            nc.vector.memset(ones_c, 1.0)
            ones_r = cpool.tile([1, 128], F32, tag="ones_r")
            nc.vector.memset(ones_r, 1.0)
            # preload the Sqrt activation table while DMAs stream
            e5s = cpool.tile([1, 1], F32, tag="e5s")
            nc.vector.memset(e5s, BN_EPS)
            scr1 = cpool.tile([1, 1], F32, tag="scr1")
            nc.scalar.activation(scr1, e5s, SQRT)

            blob1 = work.tile([128, GW + CW + 2 * TW], F32, tag="blob1")
            nc.sync.dma_start(out=blob1, in_=b1ap)
            blob2 = work.tile([128, 2 * TW], F32, tag="blob2")
            nc.sync.dma_start(out=blob2, in_=b2ap)

            gv = blob1[:, 0:GW].rearrange("p (c b j) -> p c b j", c=CPC, b=B)
            crow = blob1[:, GW:GW + CW]          # partition-replicated consts
            beta_row = crow[0:1, 0:8]
            rcpan_row = crow[0:1, 8:16]
            rcpa2n_row = crow[0:1, 16:24]
            grc_row = crow[0:1, 24:32]
            gam_row = crow[0:1, 32:40]
            eps8b = crow[:, 48:49]               # [128,1] 1e-8
            eps5r = crow[0:1, 49:50]             # [1,1] 1e-5

            def tslice(c, ty, p):
                i = (2 * ty + p) * 128
                if c < 2:
                    base = GW + CW + c * TW
                    return blob1[:, base + i:base + i + 128]
                base = (c - 2) * TW
                return blob2[:, base + i:base + i + 128]

            gsq = work.tile([128, CPC, B, Q + 1], F32, tag="gsq")
            nc.vector.tensor_mul(gsq, gv, gv)

            # ---- per-channel convs + std feature ----
            # fall[:, s, :]: seg s<4: unnormalized f_mean; s>=4: f_std
            fall = work.tile([128, NSEG, NB], F32, tag="fall")
            for c in range(CPC):
                acc1 = ps1.tile([128, NB], F32, tag="acc1")
                nc.tensor.matmul(acc1, tslice(c, 0, 0), gv[:, c, :, 0:Q],
                                 start=True, stop=False)
                nc.tensor.matmul(acc1, tslice(c, 0, 1), gv[:, c, :, 1:Q + 1],
                                 start=False, stop=True)
                acc2 = ps2.tile([128, NB], F32, tag="acc2")
                nc.tensor.matmul(acc2, tslice(c, 1, 0), gv[:, c, :, 0:Q],
                                 start=True, stop=False)
                nc.tensor.matmul(acc2, tslice(c, 1, 1), gv[:, c, :, 1:Q + 1],
                                 start=False, stop=True)
                acc3 = ps3.tile([128, NB], F32, tag="acc3")
                nc.tensor.matmul(acc3, tslice(c, 1, 0), gsq[:, c, :, 0:Q],
                                 start=True, stop=False)
                nc.tensor.matmul(acc3, tslice(c, 1, 1), gsq[:, c, :, 1:Q + 1],
                                 start=False, stop=True)

                # mean feature: raw acc to SBUF (1/s1 folds into BN affine)
                nc.vector.tensor_copy(fall[:, c, :], acc1)

                # std: v = acc3*k - (acc2*k)^2, k = 1/s2 (f_std after sqrt)
                k_ap = crow[:, 40 + c:41 + c]
                m2 = work.tile([128, NB], F32, tag="m2")
                nc.vector.tensor_scalar_mul(m2, acc2, k_ap)
                tt = work.tile([128, NB], F32, tag="tt")
                nc.vector.tensor_mul(tt, m2, m2)
                vseg = fall[:, CPC + c, :]
                nc.vector.tensor_scalar_mul(vseg, acc3, k_ap)
                nc.vector.tensor_sub(vseg, vseg, tt)
            for c in range(CPC):
                nc.scalar.activation(fall[:, CPC + c, :], fall[:, CPC + c, :],
                                     SQRT, bias=eps8b)

            # ---- BN stats: per-seg sums over (r, b, q) ----
            fsq = work.tile([128, NSEG, NB], F32, tag="fsq")
            nc.vector.tensor_mul(fsq, fall, fall)
            pack = work.tile([128, 2 * NSEG], F32, tag="pack")
            nc.vector.reduce_sum(out=pack[:, 0:NSEG], in_=fall,
                                 axis=mybir.AxisListType.X)
            nc.vector.reduce_sum(out=pack[:, NSEG:2 * NSEG], in_=fsq,
                                 axis=mybir.AxisListType.X)
            sums_ps = pss.tile([1, 2 * NSEG], F32, tag="sums")
            nc.tensor.matmul(sums_ps, ones_c, pack, start=True, stop=True)

            # ---- per-seg BN affine: a = grc*rstd, b = beta - mu*gam*rstd
            mu = work.tile([1, NSEG], F32, tag="mu")
            nc.vector.tensor_mul(mu, sums_ps[:, 0:NSEG], rcpan_row)
            msq = work.tile([1, NSEG], F32, tag="msq")
            nc.vector.tensor_mul(msq, sums_ps[:, NSEG:2 * NSEG], rcpa2n_row)
            tmp = work.tile([1, NSEG], F32, tag="tmp")
            nc.vector.tensor_mul(tmp, mu, mu)
            nc.vector.tensor_sub(msq, msq, tmp)          # biased var of f
            sq = work.tile([1, NSEG], F32, tag="sq")
            nc.scalar.activation(sq, msq, SQRT, bias=eps5r)
            rstd = work.tile([1, NSEG], F32, tag="rstd")
            nc.vector.reciprocal(rstd, sq)
            ab = work.tile([1, 2 * NSEG], F32, tag="ab")
            nc.vector.tensor_mul(ab[:, 0:NSEG], rstd, grc_row)
            gr = work.tile([1, NSEG], F32, tag="gr")
            nc.vector.tensor_mul(gr, rstd, gam_row)
            nc.vector.tensor_mul(tmp, mu, gr)
            nc.vector.tensor_sub(ab[:, NSEG:2 * NSEG], beta_row, tmp)

            # broadcast [1, 16] -> [128, 16] via K=1 matmul with ones
            abb_ps = pss.tile([128, 2 * NSEG], F32, tag="abb")
            nc.tensor.matmul(abb_ps, ones_r, ab, start=True, stop=True)
            abb = work.tile([128, 2 * NSEG], F32, tag="abbs")
            nc.vector.tensor_copy(abb, abb_ps)

            # ---- apply affine + store (one contiguous DMA) ----
            for s in range(NSEG):
                nc.vector.tensor_scalar(out=fall[:, s, :], in0=fall[:, s, :],
                                        scalar1=abb[:, s:s + 1],
                                        scalar2=abb[:, NSEG + s:NSEG + s + 1],
                                        op0=MUL, op1=ADD)
            nc.sync.dma_start(out=oap, in_=fall)

    nc.compile()
    return nc


_CACHE = {}


def _get_nc():
    if "nc" not in _CACHE:
        _CACHE["nc"] = _build_nc()
    return _CACHE["nc"]


def _host_prep(inputs):
    fs = np.ascontiguousarray(np.asarray(inputs["full_series"], np.float32))
    idx = np.asarray(inputs["indices"])
    starts = idx[:, 0].astype(np.int64)
    rows = (starts - W)[:, None] + np.arange(W + T)[None, :]
    bw = fs[rows]                                   # (B, 640, C)
    # G[c, kp, b, j] = bw[b, 128j + kp, c]
    G = bw.reshape(B, Q + 1, 128, C).transpose(3, 2, 0, 1)

    w1 = _soft_window_weights(np.asarray(inputs["raw_win_mean"], np.float64))
    w2 = _soft_window_weights(np.asarray(inputs["raw_win_std"], np.float64))
    s1 = w1.sum(axis=0)
    s2 = w2.sum(axis=0)

    gm = np.asarray(inputs["gamma_mean"], np.float64)
    bm = np.asarray(inputs["beta_mean"], np.float64)
    gs = np.asarray(inputs["gamma_std"], np.float64)
    bs = np.asarray(inputs["beta_std"], np.float64)

    in_maps = []
    for k in range(NCORES):
        ch = list(range(CPC * k, CPC * (k + 1)))
        toep = np.zeros((CPC, 2, 2, 128, 128), np.float64)
        for i, cg in enumerate(ch):
            toep[i, 0] = _toeplitz_pair(w1[:, cg])
            toep[i, 1] = _toeplitz_pair(w2[:, cg])
        rcpa = np.concatenate([1.0 / s1[ch], np.ones(CPC)])
        gam = np.concatenate([gm[ch], gs[ch]])
        cst = np.zeros(CW, np.float64)
        cst[0:8] = np.concatenate([bm[ch], bs[ch]])   # beta
        cst[8:16] = rcpa / NBT                        # rcpan
        cst[16:24] = rcpa * rcpa / NBT                # rcpa2n
        cst[24:32] = gam * rcpa                       # grc
        cst[32:40] = gam                              # gam
        cst[40:44] = 1.0 / s2[ch]                     # rcp2
        cst[48] = STD_EPS
        cst[49] = BN_EPS
        # layouts: [kp, ...]
        gpart = G[ch].transpose(1, 0, 2, 3).reshape(128, GW)
        tpart = toep.transpose(3, 0, 1, 2, 4).reshape(128, 4 * TW)
        cpart = np.broadcast_to(cst[None, :], (128, CW))
        blob1 = np.concatenate([gpart, cpart, tpart[:, 0:2 * TW]], axis=1)
        blob2 = tpart[:, 2 * TW:4 * TW]
        in_maps.append(dict(
            blob1=np.ascontiguousarray(blob1, dtype=np.float32),
            blob2=np.ascontiguousarray(blob2, dtype=np.float32),
        ))
    return in_maps


def _assemble(inputs, results):
    x = np.asarray(inputs["x"], np.float32)
    full = np.empty((B, T, 3 * C), np.float32)
    full[:, :, 0:C] = x
    for k in range(NCORES):
        o = results[k]["out"].reshape(128, 2, CPC, B, Q)
        # [r, feat, c, b, q] -> [b, q, r, c, feat] -> [b, t, c, feat]
        arr = o.transpose(3, 4, 0, 2, 1).reshape(B, T, CPC, 2)
        full[:, :, C + CPC * k:C + CPC * (k + 1)] = arr[:, :, :, 0]
        full[:, :, 2 * C + CPC * k:2 * C + CPC * (k + 1)] = arr[:, :, :, 1]
    return full


def run(inputs, trace=False):
    in_maps = _host_prep(inputs)
    nc = _get_nc()
    res = run_bass_kernel_spmd(nc, in_maps, list(range(NCORES)), trace=trace)
    return _assemble(inputs, res.results), res


def kernel(**inputs):
    out, _ = run(inputs)
    return out



# revision 13
# speedup vs baseline: 1.4793x; 1.4793x over previous
# Trainium2 Bass kernel for nn_DifferentiableFeatureLayer.
#
# Math (per reference):
#   bw[b]   = full_series[starts[b]-W : starts[b]+T]            (B, W+T, C)
#   f_mean  = conv(bw, w1)/s1 ; m2 = conv(bw, w2)/s2
#   var2    = conv(bw^2, w2)/s2 - m2^2 ; f_std = sqrt(var2 + 1e-8)
#   out     = concat([x, BN(f_mean), BN(f_std)], -1)            (B, T, 3C)
# where conv is a per-channel sliding window of length W over time and BN
# normalizes per channel over (B, T).
#
# Sharding: by channel - core k owns channels [4k, 4k+4); BN is per channel so
# cores are independent (no collectives). Host extracts the runtime-indexed
# windows and passes x through.
#
# Device compute: sliding window = banded (Toeplitz) matmul in bf16 (PSUM
# accumulates fp32):
#   acc[b, 128q+r] = sum_p sum_kp T_p[kp, r] * G[kp, b, q+p]
# The std-feature Toeplitz has 1/s2 folded in, so acc2 = m2 directly and
# acc3 = E[w2 x^2]/s2; v = acc3 - m2^2; f_std = sqrt(v + 1e-8).
# The mean feature stays in "h-units" (h = s1*f_mean): BN(h/s1) is the affine
# a*h + b with a = gamma/sqrt(var_h + s1^2*eps), b = beta - mu_h*a, so 1/s1
# only ever enters through the constant C = s1^2*eps.
#
# BN stats: per-partition partial sums (DVE reduces + fused tensor_tensor_
# reduce accumulators) -> gpsimd partition_all_reduce -> replicated [128,16]
# sums -> short per-seg affine chain -> per-seg scalars applied straight out
# of PSUM/SBUF into a bf16 output tile (DVE/ACT/Pool split).
#
# Input DMA is 3 bf16 chunks (std toeplitz+G first, then mean toeplitz,
# consts last) so std convs start while mean data is still in flight.

import numpy as np
import ml_dtypes

import concourse.bass as bass
import concourse.bacc as bacc
import concourse.tile as tile
from concourse import mybir
from concourse import bass_isa
from concourse.bass_utils import run_bass_kernel_spmd

B, T, C = 16, 512, 32
W = 128
SERIES_LEN = 100000
WIN_MIN, WIN_MAX = 2.0, 64.0
SHARP = 1.0
BN_EPS = 1e-5
STD_EPS = 1e-8

NCORES = 8
CPC = C // NCORES          # channels per core = 4
Q = T // 128               # 4 time blocks
NB = B * Q                 # 64 matmul columns
NBT = B * T                # BN population per channel
F32 = mybir.dt.float32
BF16 = mybir.dt.bfloat16
MUL = mybir.AluOpType.mult
ADD = mybir.AluOpType.add
SUB = mybir.AluOpType.subtract
SQRT = mybir.ActivationFunctionType.Sqrt
SQUARE = mybir.ActivationFunctionType.Square
IDENT = mybir.ActivationFunctionType.Identity

BNP = ml_dtypes.bfloat16

# tg blob layout (bf16, [128, 2368]):
#   chunk A1 (cols 0:672):    T1k(c0) 256 | T1k(c1) 256 | G(c0) 80 | G(c1) 80
#   chunk A2 (cols 672:1344): same for c2, c3
#   chunk B  (cols 1344:2368): T0(c0..c3), 256 each
CHUNK = 672
TGW = 2 * CHUNK + 4 * 256  # 2368


def _sigmoid(x):
    out = np.empty_like(x)
    pos = x >= 0
    out[pos] = 1.0 / (1.0 + np.exp(-x[pos]))
    ex = np.exp(x[~pos])
    out[~pos] = ex / (1.0 + ex)
    return out


def _soft_window_weights(raw):
    # (C,) -> (W, C), float64 for host-side accuracy
    win = WIN_MIN + _sigmoid(raw.astype(np.float64)) * (WIN_MAX - WIN_MIN)
    age = np.arange(W, dtype=np.float64)[::-1]
    return _sigmoid(SHARP * (win[None, :] - age[:, None]))


def _toeplitz_pair(wt):
    # wt: (W,) -> (2, 128, 128) band matrices T_p[kp, r] = wt[128p + kp - r]
    kp = np.arange(128)[:, None]
    r = np.arange(128)[None, :]
    out = np.zeros((2, 128, 128), np.float64)
    for p in range(2):
        idx = 128 * p + kp - r
        valid = (idx >= 0) & (idx < W)
        out[p] = np.where(valid, wt[np.clip(idx, 0, W - 1)], 0.0)
    return out


def _build_nc():
    nc = bacc.Bacc("TRN2", target_bir_lowering=False, debug=False,
                   num_devices=NCORES)
    tg_t = nc.dram_tensor("tg", [128, TGW], BF16, kind="ExternalInput")
    cst_t = nc.dram_tensor("cst", [128, 128], F32, kind="ExternalInput")
    out_t = nc.dram_tensor("out", [128, 8, NB], BF16, kind="ExternalOutput")
    tgap, cstap, oap = tg_t.ap(), cst_t.ap(), out_t.ap()

    with tile.TileContext(nc) as tc:
        with (
            tc.tile_pool(name="work", bufs=1) as work,
            tc.tile_pool(name="ps1", bufs=1, space="PSUM") as ps1,
            tc.tile_pool(name="ps2", bufs=1, space="PSUM") as ps2,
            tc.tile_pool(name="ps3", bufs=1, space="PSUM") as ps3,
            tc.tile_pool(name="ps4", bufs=1, space="PSUM") as ps4,
        ):
            # activation-table preload trigger (sqrt_and_others: Sqrt/Square/
            # Identity) while input DMA streams
            e5s = work.tile([1, 1], F32, tag="e5s")
            nc.vector.memset(e5s, BN_EPS)
            scr1 = work.tile([1, 1], F32, tag="scr1")
            nc.scalar.activation(scr1, e5s, SQRT)
            ones = work.tile([128, 128], F32, tag="ones")
            nc.vector.memset(ones, 1.0)

            tg = work.tile([128, TGW], BF16, tag="tg")
            nc.sync.dma_start(out=tg[:, 0:CHUNK], in_=tgap[:, 0:CHUNK])
            nc.sync.dma_start(out=tg[:, CHUNK:2 * CHUNK],
                              in_=tgap[:, CHUNK:2 * CHUNK])
            nc.sync.dma_start(out=tg[:, 2 * CHUNK:TGW],
                              in_=tgap[:, 2 * CHUNK:TGW])
            cst = work.tile([128, 128], F32, tag="cst")
            nc.sync.dma_start(out=cst, in_=cstap)

            def t1s(c, p):  # std toeplitz (k-folded)
                base = CHUNK * (c // 2) + 256 * (c % 2) + 128 * p
                return tg[:, base:base + 128]

            def t0s(c, p):  # mean toeplitz
                base = 2 * CHUNK + 256 * c + 128 * p
                return tg[:, base:base + 128]

            def gs(c):      # G(c): [128, B, Q+1]
                base = CHUNK * (c // 2) + 512 + 80 * (c % 2)
                return tg[:, base:base + 80].rearrange("p (b j) -> p b j", b=B)

            gsqt = work.tile([128, CPC, B, Q + 1], BF16, tag="gsqt")
            ttsq = work.tile([128, CPC, NB], F32, tag="ttsq")
            vt = work.tile([128, CPC, NB], F32, tag="vt")
            fstd = work.tile([128, CPC, NB], F32, tag="fstd")
            pack = work.tile([128, 16], F32, tag="pack")
            outt = work.tile([128, 8, NB], BF16, tag="outt")

            acc1 = ps1.tile([128, CPC, NB], F32, tag="acc1")
            acc2 = ps2.tile([128, CPC, NB], F32, tag="acc2")
            acc3 = ps3.tile([128, CPC, NB], F32, tag="acc3")

            # gsq per chunk (bf16, 4x DVE mode)
            for h in range(2):
                cs = slice(2 * h, 2 * h + 2)
                gv = tg[:, CHUNK * h + 512:CHUNK * h + 672].rearrange(
                    "p (c b j) -> p c b j", c=2, b=B)
                nc.vector.tensor_mul(gsqt[:, cs, :, :], gv, gv)

            # std convs (acc2 = m2, acc3 = E[w2 x^2]/s2)
            for c in range(CPC):
                g = gs(c)
                gq = gsqt[:, c, :, :]
                nc.tensor.matmul(acc2[:, c, :], t1s(c, 0), g[:, :, 0:Q],
                                 start=True, stop=False)
                nc.tensor.matmul(acc2[:, c, :], t1s(c, 1), g[:, :, 1:Q + 1],
                                 start=False, stop=True)
                nc.tensor.matmul(acc3[:, c, :], t1s(c, 0), gq[:, :, 0:Q],
                                 start=True, stop=False)
                nc.tensor.matmul(acc3[:, c, :], t1s(c, 1), gq[:, :, 1:Q + 1],
                                 start=False, stop=True)
            # mean convs
            for c in range(CPC):
                g = gs(c)
                nc.tensor.matmul(acc1[:, c, :], t0s(c, 0), g[:, :, 0:Q],
                                 start=True, stop=False)
                nc.tensor.matmul(acc1[:, c, :], t0s(c, 1), g[:, :, 1:Q + 1],
                                 start=False, stop=True)

            # ttsq = m2^2 on ACT (per channel pair), v = acc3 - m2^2 on DVE;
            # sqrt (ACT, vt->fstd) runs concurrently with the sum(v) reduce
            for h in range(2):
                cs = slice(2 * h, 2 * h + 2)
                nc.scalar.activation(ttsq[:, cs, :], acc2[:, cs, :], SQUARE)
            for h in range(2):
                cs = slice(2 * h, 2 * h + 2)
                nc.vector.tensor_sub(vt[:, cs, :], acc3[:, cs, :],
                                     ttsq[:, cs, :])
            for h in range(2):
                cs = slice(2 * h, 2 * h + 2)
                nc.scalar.activation(fstd[:, cs, :], vt[:, cs, :], SQRT,
                                     bias=cst[:, 32:33])
            nc.vector.reduce_sum(out=pack[:, 12:16], in_=vt,
                                 axis=mybir.AxisListType.X)

            # mean stats (overlap with std tail): S1 and S2 of h = acc1
            # (PSUM->SBUF copy first: HW allows only one PSUM input per op)
            hsb = work.tile([128, CPC, NB], F32, tag="hsb")
            nc.vector.tensor_copy(hsb, acc1)
            nc.vector.reduce_sum(out=pack[:, 0:4], in_=hsb,
                                 axis=mybir.AxisListType.X)
            fsq = work.tile([128, CPC, NB], F32, tag="fsq")
            nc.vector.tensor_mul(fsq, hsb, hsb)
            nc.vector.reduce_sum(out=pack[:, 8:12], in_=fsq,
                                 axis=mybir.AxisListType.X)
            nc.vector.reduce_sum(out=pack[:, 4:8], in_=fstd,
                                 axis=mybir.AxisListType.X)

            # cross-partition reduce, replicated to all partitions, via
            # all-ones stationary matmul
            sums = ps4.tile([128, 16], F32, tag="sums")
            nc.tensor.matmul(sums, ones, pack, start=True, stop=True)

            # per-seg BN affine: X = sums/N + C  (X[:,0:8]=mu, X[:,8:16]=m2c)
            # var = m2c - mu^2 ; a = gamma/sqrt(var) ; b = beta - mu*a
            X = work.tile([128, 16], F32, tag="X")
            nc.vector.scalar_tensor_tensor(
                out=X, in0=sums, scalar=1.0 / NBT, in1=cst[:, 0:16],
                op0=MUL, op1=ADD)
            tmp8 = work.tile([128, 8], F32, tag="tmp8")
            nc.vector.tensor_mul(tmp8, X[:, 0:8], X[:, 0:8])
            var8 = work.tile([128, 8], F32, tag="var8")
            nc.vector.scalar_tensor_tensor(
                out=var8, in0=tmp8, scalar=-1.0, in1=X[:, 8:16],
                op0=MUL, op1=ADD)
            sq8 = work.tile([128, 8], F32, tag="sq8")
            nc.scalar.activation(sq8, var8, SQRT)
            rstd = work.tile([128, 8], F32, tag="rstd")
            nc.vector.reciprocal(rstd, sq8)
            ab = work.tile([128, 16], F32, tag="ab")
            nc.vector.tensor_mul(ab[:, 0:8], rstd, cst[:, 16:24])
            nc.vector.tensor_mul(tmp8, X[:, 0:8], ab[:, 0:8])
            nc.vector.tensor_sub(ab[:, 8:16], cst[:, 24:32], tmp8)

            # applies: segs 0:4 mean + seg 7 std on DVE, segs 4:6 std on ACT
            for s in range(4):
                nc.vector.tensor_scalar(
                    out=outt[:, s, :], in0=hsb[:, s, :],
                    scalar1=ab[:, s:s + 1], scalar2=ab[:, 8 + s:9 + s],
                    op0=MUL, op1=ADD)
            for j in range(3):
                nc.scalar.activation(outt[:, 4 + j, :], fstd[:, j, :], IDENT,
                                     bias=ab[:, 12 + j:13 + j],
                                     scale=ab[:, 4 + j:5 + j])
            nc.vector.tensor_scalar(
                out=outt[:, 7, :], in0=fstd[:, 3, :],
                scalar1=ab[:, 7:8], scalar2=ab[:, 15:16],
                op0=MUL, op1=ADD)

            nc.sync.dma_start(out=oap, in_=outt)

    nc.compile()
    return nc


_CACHE = {}


def _get_nc():
    if "nc" not in _CACHE:
        _CACHE["nc"] = _build_nc()
    return _CACHE["nc"]


def _host_prep(inputs):
    fs = np.ascontiguousarray(np.asarray(inputs["full_series"], np.float32))
    idx = np.asarray(inputs["indices"])
    starts = idx[:, 0].astype(np.int64)
    rows = (starts - W)[:, None] + np.arange(W + T)[None, :]
    bw = fs[rows]                                   # (B, 640, C)
    # G[c, kp, b, j] = bw[b, 128j + kp, c]
    G = bw.reshape(B, Q + 1, 128, C).transpose(3, 2, 0, 1)

    w1 = _soft_window_weights(np.asarray(inputs["raw_win_mean"], np.float64))
    w2 = _soft_window_weights(np.asarray(inputs["raw_win_std"], np.float64))
    s1 = w1.sum(axis=0)
    s2 = w2.sum(axis=0)
    w2k = w2 / s2                                   # fold 1/s2 into toeplitz

    gm = np.asarray(inputs["gamma_mean"], np.float64)
    bm = np.asarray(inputs["beta_mean"], np.float64)
    gs_ = np.asarray(inputs["gamma_std"], np.float64)
    bs = np.asarray(inputs["beta_std"], np.float64)

    in_maps = []
    for k in range(NCORES):
        ch = list(range(CPC * k, CPC * (k + 1)))
        tgb = np.zeros((128, TGW), np.float64)
        for i, cg in enumerate(ch):
            t1 = _toeplitz_pair(w2k[:, cg])         # (2,128,128) [p, kp, r]
            t0 = _toeplitz_pair(w1[:, cg])
            h, m = i // 2, i % 2
            base = CHUNK * h + 256 * m
            tgb[:, base:base + 256] = t1.transpose(1, 0, 2).reshape(128, 256)
            gb = CHUNK * h + 512 + 80 * m
            tgb[:, gb:gb + 80] = G[cg].reshape(128, 80)
            b0 = 2 * CHUNK + 256 * i
            tgb[:, b0:b0 + 256] = t0.transpose(1, 0, 2).reshape(128, 256)

        cstv = np.zeros(128, np.float64)
        cstv[8:12] = s1[ch] ** 2 * BN_EPS           # C for mean segs
        cstv[12:16] = BN_EPS + STD_EPS              # C for std segs
        cstv[16:20] = gm[ch]
        cstv[20:24] = gs_[ch]
        cstv[24:28] = bm[ch]
        cstv[28:32] = bs[ch]
        cstv[32] = STD_EPS
        cpart = np.broadcast_to(cstv[None, :], (128, 128))
        in_maps.append(dict(
            tg=np.ascontiguousarray(tgb.astype(BNP)),
            cst=np.ascontiguousarray(cpart, dtype=np.float32),
        ))
    return in_maps


def _assemble(inputs, results):
    x = np.asarray(inputs["x"], np.float32)
    full = np.empty((B, T, 3 * C), np.float32)
    full[:, :, 0:C] = x
    for k in range(NCORES):
        o = np.asarray(results[k]["out"], dtype=np.float32)
        o = o.reshape(128, 2, CPC, B, Q)
        # [r, feat, c, b, q] -> [b, q, r, c, feat] -> [b, t, c, feat]
        arr = o.transpose(3, 4, 0, 2, 1).reshape(B, T, CPC, 2)
        full[:, :, C + CPC * k:C + CPC * (k + 1)] = arr[:, :, :, 0]
        full[:, :, 2 * C + CPC * k:2 * C + CPC * (k + 1)] = arr[:, :, :, 1]
    return full


def run(inputs, trace=False):
    in_maps = _host_prep(inputs)
    nc = _get_nc()
    res = run_bass_kernel_spmd(nc, in_maps, list(range(NCORES)), trace=trace)
    return _assemble(inputs, res.results), res


def kernel(**inputs):
    out, _ = run(inputs)
    return out


# revision 19
# speedup vs baseline: 1.5585x; 1.0535x over previous
# Trainium2 Bass kernel for nn_DifferentiableFeatureLayer.
#
# Math (per reference):
#   bw[b]   = full_series[starts[b]-W : starts[b]+T]            (B, W+T, C)
#   f_mean  = conv(bw, w1)/s1 ; m2 = conv(bw, w2)/s2
#   var2    = conv(bw^2, w2)/s2 - m2^2 ; f_std = sqrt(var2 + 1e-8)
#   out     = concat([x, BN(f_mean), BN(f_std)], -1)            (B, T, 3C)
# where conv is a per-channel sliding window of length W over time and BN
# normalizes per channel over (B, T).
#
# Sharding: by channel - core k owns channels [4k, 4k+4); BN is per channel so
# cores are independent (no collectives). Host extracts the runtime-indexed
# windows and passes x through.
#
# Device compute: sliding window = banded (Toeplitz) matmul in bf16 (PSUM
# accumulates fp32):
#   acc[b, 128q+r] = sum_p sum_kp T_p[kp, r] * G[kp, b, q+p]
# The std-feature Toeplitz has 1/s2 folded in, so acc2 = m2 directly and
# acc3 = E[w2 x^2]/s2; v = acc3 - m2^2; f_std = sqrt(v + 1e-8).
# The mean feature stays in "h-units" (h = s1*f_mean): BN(h/s1) is the affine
# a*h + b with a = gamma/sqrt(var_h + s1^2*eps), b = beta - mu_h*a, so 1/s1
# only ever enters through the constant C = s1^2*eps.
#
# BN stats: per-partition partial sums (DVE reduces + fused tensor_tensor_
# reduce accumulators) -> gpsimd partition_all_reduce -> replicated [128,16]
# sums -> short per-seg affine chain -> per-seg scalars applied straight out
# of PSUM/SBUF into a bf16 output tile (DVE/ACT/Pool split).
#
# Input DMA is 3 bf16 chunks (std toeplitz+G first, then mean toeplitz,
# consts last) so std convs start while mean data is still in flight.

import numpy as np
import ml_dtypes

import concourse.bass as bass
import concourse.bacc as bacc
import concourse.tile as tile
from concourse import mybir
from concourse import bass_isa
from concourse.bass_utils import run_bass_kernel_spmd

B, T, C = 16, 512, 32
W = 128
SERIES_LEN = 100000
WIN_MIN, WIN_MAX = 2.0, 64.0
SHARP = 1.0
BN_EPS = 1e-5
STD_EPS = 1e-8

NCORES = 8
CPC = C // NCORES          # channels per core = 4
Q = T // 128               # 4 time blocks
NB = B * Q                 # 64 matmul columns
NBT = B * T                # BN population per channel
F32 = mybir.dt.float32
BF16 = mybir.dt.bfloat16
MUL = mybir.AluOpType.mult
ADD = mybir.AluOpType.add
SUB = mybir.AluOpType.subtract
SQRT = mybir.ActivationFunctionType.Sqrt
SQUARE = mybir.ActivationFunctionType.Square
IDENT = mybir.ActivationFunctionType.Identity

BNP = ml_dtypes.bfloat16

# tg blob layout (bf16, [128, 2368]):
#   chunk A1 (cols 0:672):    T1k(c0) 256 | T1k(c1) 256 | G(c0) 80 | G(c1) 80
#   chunk A2 (cols 672:1344): same for c2, c3
#   chunk B  (cols 1344:2368): T0(c0..c3), 256 each
CHUNK = 672
TGW = 2 * CHUNK + 4 * 256  # 2368


def _sigmoid(x):
    out = np.empty_like(x)
    pos = x >= 0
    out[pos] = 1.0 / (1.0 + np.exp(-x[pos]))
    ex = np.exp(x[~pos])
    out[~pos] = ex / (1.0 + ex)
    return out


def _soft_window_weights(raw):
    # (C,) -> (W, C), float64 for host-side accuracy
    win = WIN_MIN + _sigmoid(raw.astype(np.float64)) * (WIN_MAX - WIN_MIN)
    age = np.arange(W, dtype=np.float64)[::-1]
    return _sigmoid(SHARP * (win[None, :] - age[:, None]))


def _toeplitz_pair(wt):
    # wt: (W,) -> (2, 128, 128) band matrices T_p[kp, r] = wt[128p + kp - r]
    kp = np.arange(128)[:, None]
    r = np.arange(128)[None, :]
    out = np.zeros((2, 128, 128), np.float64)
    for p in range(2):
        idx = 128 * p + kp - r
        valid = (idx >= 0) & (idx < W)
        out[p] = np.where(valid, wt[np.clip(idx, 0, W - 1)], 0.0)
    return out


def _build_nc():
    nc = bacc.Bacc("TRN2", target_bir_lowering=False, debug=False,
                   num_devices=NCORES)
    tg_t = nc.dram_tensor("tg", [128, TGW], BF16, kind="ExternalInput")
    cst_t = nc.dram_tensor("cst", [128, 128], F32, kind="ExternalInput")
    out_t = nc.dram_tensor("out", [128, 8, NB], BF16, kind="ExternalOutput")
    tgap, cstap, oap = tg_t.ap(), cst_t.ap(), out_t.ap()

    with tile.TileContext(nc) as tc:
        with (
            tc.tile_pool(name="work", bufs=1) as work,
            tc.tile_pool(name="ps1", bufs=1, space="PSUM") as ps1,
            tc.tile_pool(name="ps2", bufs=1, space="PSUM") as ps2,
            tc.tile_pool(name="ps3", bufs=1, space="PSUM") as ps3,
            tc.tile_pool(name="ps4", bufs=1, space="PSUM") as ps4,
        ):
            # activation-table preload trigger (sqrt_and_others: Sqrt/Square/
            # Identity) while input DMA streams
            e5s = work.tile([1, 1], F32, tag="e5s")
            nc.vector.memset(e5s, BN_EPS)
            scr1 = work.tile([1, 1], F32, tag="scr1")
            nc.scalar.activation(scr1, e5s, SQRT)
            ones = work.tile([128, 128], F32, tag="ones")
            nc.vector.memset(ones, 1.0)

            tg = work.tile([128, TGW], BF16, tag="tg")
            nc.sync.dma_start(out=tg[:, 0:CHUNK], in_=tgap[:, 0:CHUNK])
            nc.sync.dma_start(out=tg[:, CHUNK:2 * CHUNK],
                              in_=tgap[:, CHUNK:2 * CHUNK])
            nc.sync.dma_start(out=tg[:, 2 * CHUNK:TGW],
                              in_=tgap[:, 2 * CHUNK:TGW])
            cst = work.tile([128, 128], F32, tag="cst")
            nc.sync.dma_start(out=cst, in_=cstap)

            def t1s(c, p):  # std toeplitz (k-folded)
                base = CHUNK * (c // 2) + 256 * (c % 2) + 128 * p
                return tg[:, base:base + 128]

            def t0s(c, p):  # mean toeplitz
                base = 2 * CHUNK + 256 * c + 128 * p
                return tg[:, base:base + 128]

            def gs(c):      # G(c): [128, B, Q+1]
                base = CHUNK * (c // 2) + 512 + 80 * (c % 2)
                return tg[:, base:base + 80].rearrange("p (b j) -> p b j", b=B)

            gsqt = work.tile([128, CPC, B, Q + 1], BF16, tag="gsqt")
            ttsq = work.tile([128, CPC, NB], F32, tag="ttsq")
            vt = work.tile([128, CPC, NB], F32, tag="vt")
            fstd = work.tile([128, CPC, NB], F32, tag="fstd")
            pack = work.tile([128, 16], F32, tag="pack")
            outt = work.tile([128, 8, NB], BF16, tag="outt")

            # per-channel-pair PSUM tiles so pair-01 consumers don't wait on
            # pair-23 conv writers (whole-tile dependency granularity)
            acc1p = [ps1.tile([128, 2, NB], F32, name=f"acc1{h}",
                               tag=f"acc1{h}") for h in range(2)]
            acc2p = [ps2.tile([128, 2, NB], F32, name=f"acc2{h}",
                               tag=f"acc2{h}") for h in range(2)]
            acc3p = [ps3.tile([128, 2, NB], F32, name=f"acc3{h}",
                               tag=f"acc3{h}") for h in range(2)]

            # gsq per chunk (bf16, 4x DVE mode)
            for h in range(2):
                cs = slice(2 * h, 2 * h + 2)
                gv = tg[:, CHUNK * h + 512:CHUNK * h + 672].rearrange(
                    "p (c b j) -> p c b j", c=2, b=B)
                nc.vector.tensor_mul(gsqt[:, cs, :, :], gv, gv)

            # std convs (acc2 = m2, acc3 = E[w2 x^2]/s2)
            for c in range(CPC):
                g = gs(c)
                gq = gsqt[:, c, :, :]
                a2 = acc2p[c // 2][:, c % 2, :]
                a3 = acc3p[c // 2][:, c % 2, :]
                nc.tensor.matmul(a2, t1s(c, 0), g[:, :, 0:Q],
                                 start=True, stop=False)
                nc.tensor.matmul(a2, t1s(c, 1), g[:, :, 1:Q + 1],
                                 start=False, stop=True)
                nc.tensor.matmul(a3, t1s(c, 0), gq[:, :, 0:Q],
                                 start=True, stop=False)
                nc.tensor.matmul(a3, t1s(c, 1), gq[:, :, 1:Q + 1],
                                 start=False, stop=True)
            # mean convs
            for c in range(CPC):
                g = gs(c)
                a1 = acc1p[c // 2][:, c % 2, :]
                nc.tensor.matmul(a1, t0s(c, 0), g[:, :, 0:Q],
                                 start=True, stop=False)
                nc.tensor.matmul(a1, t0s(c, 1), g[:, :, 1:Q + 1],
                                 start=False, stop=True)

            # ttsq = m2^2 on ACT (per channel pair), v = acc3 - m2^2 on DVE;
            # sqrt (ACT, vt->fstd) runs concurrently with the sum(v) reduce
            for h in range(2):
                cs = slice(2 * h, 2 * h + 2)
                nc.scalar.activation(ttsq[:, cs, :], acc2p[h], SQUARE)
            for h in range(2):
                cs = slice(2 * h, 2 * h + 2)
                nc.vector.tensor_sub(vt[:, cs, :], acc3p[h], ttsq[:, cs, :])
            for h in range(2):
                cs = slice(2 * h, 2 * h + 2)
                nc.scalar.activation(fstd[:, cs, :], vt[:, cs, :], SQRT,
                                     bias=cst[:, 32:33])
            nc.vector.reduce_sum(out=pack[:, 12:16], in_=vt,
                                 axis=mybir.AxisListType.X)

            # mean stats (overlap with std tail): S1 via DVE reduce straight
            # from PSUM; S2 via ACT Square (h^2 -> SBUF) + DVE reduce
            fsq = work.tile([128, CPC, NB], F32, tag="fsq")
            for h in range(2):
                cs = slice(2 * h, 2 * h + 2)
                nc.scalar.activation(fsq[:, cs, :], acc1p[h], SQUARE)
            for h in range(2):
                nc.vector.reduce_sum(out=pack[:, 2 * h:2 * h + 2],
                                     in_=acc1p[h], axis=mybir.AxisListType.X)
            nc.vector.reduce_sum(out=pack[:, 8:12], in_=fsq,
                                 axis=mybir.AxisListType.X)
            nc.vector.reduce_sum(out=pack[:, 4:8], in_=fstd,
                                 axis=mybir.AxisListType.X)

            # cross-partition reduce, replicated to all partitions, via
            # all-ones stationary matmul
            sums = ps4.tile([128, 16], F32, tag="sums")
            nc.tensor.matmul(sums, ones, pack, start=True, stop=True)

            # per-seg BN affine: X = sums/N + C  (X[:,0:8]=mu, X[:,8:16]=m2c)
            # var = m2c - mu^2 ; a = gamma/sqrt(var) ; b = beta - mu*a
            X = work.tile([128, 16], F32, tag="X")
            nc.vector.scalar_tensor_tensor(
                out=X, in0=sums, scalar=1.0 / NBT, in1=cst[:, 0:16],
                op0=MUL, op1=ADD)
            tmp8 = work.tile([128, 8], F32, tag="tmp8")
            nc.vector.tensor_mul(tmp8, X[:, 0:8], X[:, 0:8])
            var8 = work.tile([128, 8], F32, tag="var8")
            nc.vector.scalar_tensor_tensor(
                out=var8, in0=tmp8, scalar=-1.0, in1=X[:, 8:16],
                op0=MUL, op1=ADD)
            sq8 = work.tile([128, 8], F32, tag="sq8")
            nc.scalar.activation(sq8, var8, SQRT)
            rstd = work.tile([128, 8], F32, tag="rstd")
            nc.vector.reciprocal(rstd, sq8)
            ab = work.tile([128, 16], F32, tag="ab")
            nc.vector.tensor_mul(ab[:, 0:8], rstd, cst[:, 16:24])
            nc.vector.tensor_mul(tmp8, X[:, 0:8], ab[:, 0:8])
            nc.vector.tensor_sub(ab[:, 8:16], cst[:, 24:32], tmp8)

            # applies: segs 0:4 mean (from PSUM) + seg 4 std on DVE,
            # segs 5:7 std on ACT
            for s in range(4):
                nc.vector.tensor_scalar(
                    out=outt[:, s, :], in0=acc1p[s // 2][:, s % 2, :],
                    scalar1=ab[:, s:s + 1], scalar2=ab[:, 8 + s:9 + s],
                    op0=MUL, op1=ADD)
            nc.vector.tensor_scalar(
                out=outt[:, 4, :], in0=fstd[:, 0, :],
                scalar1=ab[:, 4:5], scalar2=ab[:, 12:13],
                op0=MUL, op1=ADD)
            for j in range(1, 4):
                nc.scalar.activation(outt[:, 4 + j, :], fstd[:, j, :], IDENT,
                                     bias=ab[:, 12 + j:13 + j],
                                     scale=ab[:, 4 + j:5 + j])

            nc.sync.dma_start(out=oap, in_=outt)

    nc.compile()
    return nc


_CACHE = {}


def _get_nc():
    if "nc" not in _CACHE:
        _CACHE["nc"] = _build_nc()
    return _CACHE["nc"]


def _host_prep(inputs):
    fs = np.ascontiguousarray(np.asarray(inputs["full_series"], np.float32))
    idx = np.asarray(inputs["indices"])
    starts = idx[:, 0].astype(np.int64)
    rows = (starts - W)[:, None] + np.arange(W + T)[None, :]
    bw = fs[rows]                                   # (B, 640, C)
    # G[c, kp, b, j] = bw[b, 128j + kp, c]
    G = bw.reshape(B, Q + 1, 128, C).transpose(3, 2, 0, 1)

    w1 = _soft_window_weights(np.asarray(inputs["raw_win_mean"], np.float64))
    w2 = _soft_window_weights(np.asarray(inputs["raw_win_std"], np.float64))
    s1 = w1.sum(axis=0)
    s2 = w2.sum(axis=0)
    w2k = w2 / s2                                   # fold 1/s2 into toeplitz

    gm = np.asarray(inputs["gamma_mean"], np.float64)
    bm = np.asarray(inputs["beta_mean"], np.float64)
    gs_ = np.asarray(inputs["gamma_std"], np.float64)
    bs = np.asarray(inputs["beta_std"], np.float64)

    in_maps = []
    for k in range(NCORES):
        ch = list(range(CPC * k, CPC * (k + 1)))
        tgb = np.zeros((128, TGW), np.float64)
        for i, cg in enumerate(ch):
            t1 = _toeplitz_pair(w2k[:, cg])         # (2,128,128) [p, kp, r]
            t0 = _toeplitz_pair(w1[:, cg])
            h, m = i // 2, i % 2
            base = CHUNK * h + 256 * m
            tgb[:, base:base + 256] = t1.transpose(1, 0, 2).reshape(128, 256)
            gb = CHUNK * h + 512 + 80 * m
            tgb[:, gb:gb + 80] = G[cg].reshape(128, 80)
            b0 = 2 * CHUNK + 256 * i
            tgb[:, b0:b0 + 256] = t0.transpose(1, 0, 2).reshape(128, 256)

        cstv = np.zeros(128, np.float64)
        cstv[8:12] = s1[ch] ** 2 * BN_EPS           # C for mean segs
        cstv[12:16] = BN_EPS + STD_EPS              # C for std segs
        cstv[16:20] = gm[ch]
        cstv[20:24] = gs_[ch]
        cstv[24:28] = bm[ch]
        cstv[28:32] = bs[ch]
        cstv[32] = STD_EPS
        cpart = np.broadcast_to(cstv[None, :], (128, 128))
        in_maps.append(dict(
            tg=np.ascontiguousarray(tgb.astype(BNP)),
            cst=np.ascontiguousarray(cpart, dtype=np.float32),
        ))
    return in_maps


def _assemble(inputs, results):
    x = np.asarray(inputs["x"], np.float32)
    full = np.empty((B, T, 3 * C), np.float32)
    full[:, :, 0:C] = x
    for k in range(NCORES):
        o = np.asarray(results[k]["out"], dtype=np.float32)
        o = o.reshape(128, 2, CPC, B, Q)
        # [r, feat, c, b, q] -> [b, q, r, c, feat] -> [b, t, c, feat]
        arr = o.transpose(3, 4, 0, 2, 1).reshape(B, T, CPC, 2)
        full[:, :, C + CPC * k:C + CPC * (k + 1)] = arr[:, :, :, 0]
        full[:, :, 2 * C + CPC * k:2 * C + CPC * (k + 1)] = arr[:, :, :, 1]
    return full


def run(inputs, trace=False):
    in_maps = _host_prep(inputs)
    nc = _get_nc()
    res = run_bass_kernel_spmd(nc, in_maps, list(range(NCORES)), trace=trace)
    return _assemble(inputs, res.results), res


def kernel(**inputs):
    out, _ = run(inputs)
    return out
